# revision 23
# baseline (speedup 1.0000x reference)
"""AFMoE attention layer on 8 NeuronCores (Trainium2, Bass/Tile).

Sharding: core c = (batch b = c//4) x (kv-head group g = c%4).
Each core computes its batch's q-heads 4g..4g+3 + kv head g end-to-end and a
partial output y_c = O_gated @ Wo[:, 512g:512(g+1)].T; the host sums the 4
group partials per batch (row-parallel Wo reduction done on host).

Structure (v3):
  phase A: qkv projection + rms-norm + rope + transposes (per 128-s-tile).
           RMS sums via scalar Square+accum, rope adds on DVE, coarse DMAs
           (the HWDGE issue cost ~0.7us/instr was the phase-A bottleneck).
  phase B: attention; gate projection interleaved per s-quarter, 2-head-
           batched score/PV/rowsum matmuls (N=512), post-exp binary mask
           multiplies on DVE, rowsums accumulated at partitions {0,32} and
           inverted directly on the scalar engine, one N=512 broadcast
           matmul per head-pair, Wo emission lagging one pair behind.
"""
import os

import numpy as np

import concourse.bass as bass
import concourse.mybir as mybir
import concourse.tile as tile
from concourse.bass_utils import run_bass_kernel_spmd
from concourse.masks import make_identity

F32 = mybir.dt.float32
F32R = mybir.dt.float32r
BF16 = mybir.dt.bfloat16
FP16 = mybir.dt.float16
AF = mybir.ActivationFunctionType
ALU = mybir.AluOpType
AX = mybir.AxisListType

B, S, H = 2, 2048, 2048
NH, NKV, D = 16, 4, 128
GROUPS = NH // NKV          # q heads per kv head = 4
QH = GROUPS                 # per-core q heads
DQ = QH * D                 # 512
EPS = 1e-5
NT = S // 128               # 16 s-tiles
HC = H // 128               # 16 h-chunks
LAM = float(D) ** -0.5
SQ = S // 4                 # 512 per s-quarter

_nsplit = [0]


def _split_excess_waits(nc, limit=1):
    """This walrus build accepts only one semaphore wait per instruction
    (fp32/fp32r matmuls included). Move excess waits onto preceding
    same-engine NoOps; engine program order keeps this correct."""
    import bass_rust
    for blk in nc.m.functions[0].blocks:
        lst = blk.instructions
        idx = 0
        while idx < len(lst):
            inst = lst[idx]
            si = inst.sync_info
            if (si is None or len(si.on_wait) <= limit
                    or type(inst).__name__ == "InstCollectiveCompute"
                    or inst.engine == mybir.EngineType.Unassigned):
                idx += 1
                continue
            waits = list(si.on_wait)
            kept, excess = waits[-limit:], waits[:-limit]
            new_insts = []
            for w in excess:
                _nsplit[0] += 1
                nop = mybir.InstNoOp(name=f"WS-{_nsplit[0]}", ins=[], outs=[])
                nop.engine = inst.engine
                nop.sync_info = bass_rust.SyncInfo(on_wait=[w], on_update=[])
                new_insts.append(nop)
            inst.sync_info = bass_rust.SyncInfo(on_wait=kept,
                                                on_update=list(si.on_update))
            lst[idx:idx] = new_insts
            idx += len(new_insts) + 1


def _scalar_recip(nc, out, in_):
    """Reciprocal on the scalar engine (bass guards this off for accuracy;
    the ~1e-3 level error is fine for this kernel's 2e-2 budget)."""
    eng = nc.scalar
    inputs = [
        eng.lower_ap(in_),
        mybir.ImmediateValue(dtype=mybir.dt.float32, value=0.0),
        mybir.ImmediateValue(dtype=mybir.dt.float32, value=1.0),
        mybir.ImmediateValue(dtype=mybir.dt.float32, value=0.0),
    ]
    outputs = [eng.lower_ap(out)]
    return eng.add_instruction(
        mybir.InstActivation(
            name=nc.get_next_instruction_name(),
            func=AF.Reciprocal,
            ins=inputs,
            outs=outputs,
        ))


def _mask_plan(mask2d):
    """Classify the additive mask in [256(q) x 128(k)] slabs (q-tile pairs).

    Returns (rows, mixed_slabs): rows[pair] = list of (kj, mixed_idx|None)
    over a contiguous kj range; mixed_slabs = transposed [128,256] np arrays
    holding BINARY (1.0 allowed / 0.0 masked) values.
    """
    nb = S // 128
    npair = nb // 2
    uniq = {}
    mixed = []
    rows = []

    def binmask(blk):
        key = blk.tobytes()
        if key not in uniq:
            uniq[key] = len(mixed)
            mixed.append(
                np.ascontiguousarray((blk.T > -1e8).astype(np.float32)))
        return uniq[key]

    for p in range(npair):
        qsl = slice(p * 256, (p + 1) * 256)
        entries = []
        for kj in range(nb):
            blk = mask2d[qsl, kj * 128:(kj + 1) * 128]      # [256 q, 128 k]
            if (blk <= -1e8).all():
                entries.append(None)
            elif (blk == 0.0).all():
                entries.append((kj, None))
            else:
                entries.append((kj, binmask(blk)))
        live = [e for e in entries if e is not None]
        if not live:
            raise ValueError("fully-masked query row block unsupported")
        lo = min(e[0] for e in live)
        hi = max(e[0] for e in live)
        row = []
        for kj in range(lo, hi + 1):
            e = entries[kj]
            if e is None:
                blk = mask2d[qsl, kj * 128:(kj + 1) * 128]
                row.append((kj, binmask(blk)))
            else:
                row.append(e)
        rows.append(row)
    return rows, mixed


def _build(rows, nmix):
    nc = bass.Bass()
    wqkv = nc.declare_dram_parameter("wqkv", [H, DQ + 2 * D], FP16, isOutput=False)
    xt16 = nc.declare_dram_parameter("xt16", [H, S], FP16, isOutput=False)
    wg = nc.declare_dram_parameter("wg", [H, DQ], FP16, isOutput=False)
    wo = nc.declare_dram_parameter("wo", [DQ, H], FP16, isOutput=False)
    csw = nc.declare_dram_parameter("csw", [S, 4 * D], FP16, isOutput=False)
    if nmix:
        maskt = nc.declare_dram_parameter("maskt", [nmix * 128, 256], BF16,
                                          isOutput=False)
    y = nc.declare_dram_parameter("y", [S, H], F32, isOutput=True)

    NW = DQ + 2 * D  # 768

    with tile.TileContext(nc) as tc, \
            nc.allow_low_precision(reason="fp32r matmul operands"), \
            tc.tile_pool(name="const", bufs=1) as const, \
            tc.tile_pool(name="persist", bufs=1) as pp, \
            tc.tile_pool(name="pwg", bufs=1) as pwg, \
            tc.tile_pool(name="pxtb", bufs=1) as pxtb:
        identity_f = const.tile([128, 128], F32)
        make_identity(nc, identity_f)
        identity_h = const.tile([128, 128], FP16)
        nc.vector.tensor_copy(identity_h, identity_f)
        ones_f = const.tile([128, 128], F32)
        nc.vector.memset(ones_f, 1.0)
        ones128 = const.tile([128, 128], BF16)   # rowsum-with-broadcast lhsT
        nc.vector.tensor_copy(ones128, ones_f)
        eps_t = const.tile([128, 1], F32)
        nc.vector.memset(eps_t, EPS)
        ebias_t = const.tile([128, 1], F32)
        nc.vector.memset(ebias_t, -2.0)

        qT_all = pp.tile([128, 2, NT // 2, 2, 256], FP16)     # [d, h, s]
        kT_all = pp.tile([128, S], FP16)         # [d, s]
        v_all = pp.tile([128, NT, D], BF16)      # [s-part, s-tile, d]

        wg_sb = pwg.tile([128, HC, DQ], FP16)
        # gate sigmoids, ring of 2 s-quarters
        sig_ring = pwg.tile([128, QH, 2, SQ], FP16)
        # unnormalized gated attention out, ring of 2 pairs
        otg_ring = pwg.tile([128, QH, 2, 256], FP16)
        xtb = pxtb.tile([128, HC, SQ], FP16)     # gate activations, 1 quarter

        wg2 = wg.rearrange("(c p) w -> p c w", p=128)
        xtq = xt16.rearrange("(c p) (s q) -> p c s q", p=128, s=4)

        def load_xtb(sh):
            nc.sync.dma_start(out=xtb, in_=xtq[:, :, sh, :])

        def emit_gate_with(sh, mkpg):
            """Gate projection for s-quarter sh into sig_ring[sh%2].
            Adjacent m-blocks go to different PSUM banks so consecutive
            matmuls never target the same bank."""
            for m0 in (0, 2):
                pga = mkpg()
                pgb = mkpg()
                for h in range(HC):
                    nc.tensor.matmul(
                        pga[:, :512], wg_sb[:, h, m0 * 128:(m0 + 1) * 128],
                        xtb[:, h, :],
                        start=(h == 0), stop=(h == HC - 1))
                    nc.tensor.matmul(
                        pgb[:, :512],
                        wg_sb[:, h, (m0 + 1) * 128:(m0 + 2) * 128],
                        xtb[:, h, :],
                        start=(h == 0), stop=(h == HC - 1))
                nc.scalar.activation(sig_ring[:, m0, sh % 2, :],
                                     pga[:, :512], AF.Sigmoid)
                nc.scalar.activation(sig_ring[:, m0 + 1, sh % 2, :],
                                     pgb[:, :512], AF.Sigmoid)
            if sh + 1 < 4:
                load_xtb(sh + 1)

        # ---------------- phase A: q/k/v projections + norm + rope ----------
        with tc.tile_pool(name="pwq", bufs=1) as pwq, \
                tc.tile_pool(name="pa", bufs=2) as pa, \
                tc.tile_pool(name="psa", bufs=2, space="PSUM") as psa:
            wqkv_sb = pwq.tile([128, HC, NW], FP16)
            xt4 = xt16.rearrange("(c p) (t q) -> p c t q", p=128, q=128)
            wqkv8 = wqkv.rearrange("(c f p) w -> p c f w", p=128, f=2)
            csw2 = csw.rearrange("(t f p) d -> p t f d", p=128, f=2)
            ropes = {}
            csw_t = None

            def emit_transpose(st):
                qrope, krope = ropes.pop(st)
                sl = slice(st * 128, (st + 1) * 128)
                ptq = psa.tile([128, QH, 128], FP16, tag="ptq", bufs=2)
                for h in range(QH):
                    nc.tensor.transpose(ptq[:, h, :], qrope[:, h, :],
                                        identity_h)
                ptk = psa.tile([128, 128], FP16, tag="ptk", bufs=2)
                nc.tensor.transpose(ptk, krope, identity_h)
                nc.scalar.copy(
                    qT_all[:, :, st // 2, :,
                           (st % 2) * 128:(st % 2) * 128 + 128],
                    ptq.rearrange("p (a u) q -> p a u q", a=2))
                nc.scalar.copy(kT_all[:, sl], ptk)

            for st in range(NT):
                xt_t = pa.tile([128, HC, 128], FP16, tag="xt", bufs=3)
                if st == 0:
                    nc.sync.dma_start(out=xt_t[:, 0:2, :],
                                      in_=xt4[:, 0:2, st, :])
                    nc.sync.dma_start(out=wqkv_sb[:, 0:2, :],
                                      in_=wqkv8[:, 0, :, :])
                    nc.sync.dma_start(out=xt_t[:, 2:8, :],
                                      in_=xt4[:, 2:8, st, :])
                    nc.sync.dma_start(out=xt_t[:, 8:16, :],
                                      in_=xt4[:, 8:16, st, :])
                    for c8 in range(1, 8):
                        nc.sync.dma_start(
                            out=wqkv_sb[:, 2 * c8:2 * c8 + 2, :],
                            in_=wqkv8[:, c8, :, :])
                else:
                    nc.sync.dma_start(out=xt_t[:, 0:8, :],
                                      in_=xt4[:, 0:8, st, :])
                    nc.sync.dma_start(out=xt_t[:, 8:16, :],
                                      in_=xt4[:, 8:16, st, :])
                if st % 2 == 0:
                    csw_t = pa.tile([128, 2, 4 * D], FP16, tag="csw")
                    nc.sync.dma_start(out=csw_t, in_=csw2[:, st // 2])
                if st == 6:
                    nc.sync.dma_start(out=wg_sb[:, 0:8, :],
                                      in_=wg2[:, 0:8, :])
                if st == 8:
                    nc.sync.dma_start(out=wg_sb[:, 8:16, :],
                                      in_=wg2[:, 8:16, :])
                if st == 8:
                    load_xtb(0)
                cwq_t = csw_t[:, st % 2, 0 * D:1 * D]
                swq_t = csw_t[:, st % 2, 1 * D:2 * D]
                cwk_t = csw_t[:, st % 2, 2 * D:3 * D]
                swk_t = csw_t[:, st % 2, 3 * D:4 * D]

                pqkv = psa.tile([128, NW], F32, tag="pqkv", bufs=2)
                for h in range(HC):
                    nc.tensor.matmul(pqkv[:, :DQ], xt_t[:, h, :],
                                     wqkv_sb[:, h, :DQ],
                                     start=(h == 0), stop=(h == HC - 1))
                    nc.tensor.matmul(pqkv[:, DQ:], xt_t[:, h, :],
                                     wqkv_sb[:, h, DQ:],
                                     start=(h == 0), stop=(h == HC - 1))
                q_raw = pa.tile([128, DQ], F32, tag="qraw")
                nc.scalar.copy(q_raw, pqkv[:, :DQ])
                k_raw = pa.tile([128, D], F32, tag="kraw")
                nc.scalar.copy(k_raw, pqkv[:, DQ:DQ + D])
                nc.scalar.copy(v_all[:, st, :], pqkv[:, DQ + D:])

                # rms-norm sums on the scalar engine (Square + accumulate)
                sq = pa.tile([128, D], F32, tag="sq")
                ssq = pa.tile([128, QH + 1], F32, tag="ssq")
                for h in range(QH):
                    nc.scalar.activation(sq, q_raw[:, h * D:(h + 1) * D],
                                         AF.Square,
                                         accum_out=ssq[:, h:h + 1])
                nc.scalar.activation(sq, k_raw, AF.Square,
                                     accum_out=ssq[:, QH:QH + 1])
                rtq = pa.tile([128, QH + 1], F32, tag="rtq")
                nc.scalar.activation(rtq, ssq, AF.Sqrt, bias=eps_t,
                                     scale=1.0 / D)
                rq = pa.tile([128, QH + 1], F32, tag="rq")
                nc.vector.reciprocal(rq, rtq)

                # rope swaps (half-rotations) of the raw values
                r_q = pa.tile([128, QH, D], F32, tag="rqrot")
                qv = q_raw.rearrange("p (h s d) -> p h s d", h=QH, s=2)
                rv = r_q.rearrange("p h (s d) -> p h s d", s=2)
                nc.gpsimd.tensor_copy(out=rv[:, :, 0, :], in_=qv[:, :, 1, :])
                nc.gpsimd.tensor_copy(out=rv[:, :, 1, :], in_=qv[:, :, 0, :])
                r_k = pa.tile([128, D], F32, tag="rkrot")
                nc.gpsimd.tensor_copy(out=r_k[:, :64], in_=k_raw[:, 64:])
                nc.gpsimd.tensor_copy(out=r_k[:, 64:], in_=k_raw[:, :64])

                qrope = pa.tile([128, QH, D], FP16, tag="qrope", bufs=3)
                qh = q_raw.rearrange("p (h d) -> p h d", d=D)
                for h in range(QH):
                    nc.vector.scalar_tensor_tensor(
                        qrope[:, h, :], qh[:, h, :], rq[:, h:h + 1], cwq_t,
                        op0=ALU.mult, op1=ALU.mult)
                    nc.vector.scalar_tensor_tensor(
                        r_q[:, h, :], r_q[:, h, :], rq[:, h:h + 1], swq_t,
                        op0=ALU.mult, op1=ALU.mult)
                nc.vector.tensor_add(qrope, qrope, r_q)

                krope = pa.tile([128, D], FP16, tag="krope", bufs=3)
                nc.vector.scalar_tensor_tensor(krope, k_raw,
                                               rq[:, QH:QH + 1], cwk_t,
                                               op0=ALU.mult, op1=ALU.mult)
                nc.vector.scalar_tensor_tensor(r_k, r_k, rq[:, QH:QH + 1],
                                               swk_t,
                                               op0=ALU.mult, op1=ALU.mult)
                nc.vector.tensor_add(krope, krope, r_k)

                ropes[st] = (qrope, krope)
                if st >= 2:
                    emit_transpose(st - 2)
            emit_gate_with(0, lambda: psa.tile(
                [128, NW], F32, tag="pqkv", bufs=2, name="pg0"))
            emit_transpose(NT - 2)
            emit_transpose(NT - 1)

        # ---------------- phase B: attention + gating + Wo ------------------
        with tc.tile_pool(name="atw", bufs=1) as atw, \
                tc.tile_pool(name="at", bufs=2) as at, \
                tc.tile_pool(name="po", bufs=2) as po, \
                tc.tile_pool(name="ps_fx", bufs=2, space="PSUM") as ps_fx, \
                tc.tile_pool(name="ps_ot", bufs=2, space="PSUM") as ps_ot, \
                tc.tile_pool(name="ps_rs", bufs=2, space="PSUM") as ps_rs, \
                tc.tile_pool(name="ps_y", bufs=2, space="PSUM") as ps_y:
            wo_sb = atw.tile([128, QH, H], FP16)
            wo2 = wo.rearrange("(c p) w -> p c w", p=128)
            nc.sync.dma_start(out=wo_sb[:, 0:2, :], in_=wo2[:, 0:2, :])
            nc.sync.dma_start(out=wo_sb[:, 2:4, :], in_=wo2[:, 2:4, :])
            if nmix:
                maskt_sb = atw.tile([128, nmix, 256], BF16)
                mt = maskt.rearrange("(m p) q -> p m q", p=128)
                nc.sync.dma_start(out=maskt_sb, in_=mt)


            def emit_gate(sh):
                emit_gate_with(sh, lambda: ps_y.tile(
                    [128, 512], F32, tag="py", name="pg"))

            def emit_wo(pr, final=False):
                slot = pr % 2
                for sti in range(2):
                    st = 2 * pr + sti
                    y_sb = po.tile([128, H], F32, tag="ysb")
                    for n4 in (0, 2):
                        pya = ps_y.tile([128, 512], F32, tag="py")
                        pyb = ps_y.tile([128, 512], F32, tag="py")
                        for dc in range(QH):
                            lhs = otg_ring[:, dc, slot,
                                           sti * 128:(sti + 1) * 128]
                            nc.tensor.matmul(
                                pya, lhs,
                                wo_sb[:, dc, n4 * 512:(n4 + 1) * 512],
                                start=(dc == 0), stop=(dc == QH - 1))
                            nc.tensor.matmul(
                                pyb, lhs,
                                wo_sb[:, dc, (n4 + 1) * 512:(n4 + 2) * 512],
                                start=(dc == 0), stop=(dc == QH - 1))
                        nc.vector.tensor_copy(
                            y_sb[:, n4 * 512:(n4 + 1) * 512], pya)
                        if final:
                            nc.sync.dma_start(
                                out=y[st * 128:(st + 1) * 128,
                                      n4 * 512:(n4 + 1) * 512],
                                in_=y_sb[:, n4 * 512:(n4 + 1) * 512])
                        nc.vector.tensor_copy(
                            y_sb[:, (n4 + 1) * 512:(n4 + 2) * 512], pyb)
                        if final:
                            nc.sync.dma_start(
                                out=y[st * 128:(st + 1) * 128,
                                      (n4 + 1) * 512:(n4 + 2) * 512],
                                in_=y_sb[:, (n4 + 1) * 512:(n4 + 2) * 512])
                    if not final:
                        nc.sync.dma_start(
                            out=y[st * 128:(st + 1) * 128, :], in_=y_sb)

            for pr in range(NT // 2):
                if pr % 2 == 0 and pr > 0:
                    emit_gate(pr // 2)
                row = rows[pr]
                qsl = slice(pr * 256, (pr + 1) * 256)
                slot = pr % 2
                ot_ps = []
                rs_ps = []
                for _hh in range(2):
                    ot_tile = ps_ot.tile([128, 512], F32, tag="ot")
                    ot_ps.append(ot_tile)
                    rs_tile = ps_rs.tile([128, 512], F32, tag="rs")
                    rs_ps.append(rs_tile)
                nk = len(row)

                def flush_pv(ki, kj, ests):
                    for hh in range(2):
                        est = ests[hh]
                        nc.tensor.matmul(
                            ot_ps[hh], v_all[:, kj, :], est,
                            start=(ki == 0), stop=(ki == nk - 1))
                        nc.tensor.matmul(
                            rs_ps[hh], ones128, est,
                            start=(ki == 0), stop=(ki == nk - 1))

                # scores+exp run one k-block ahead of PV/rowsum so the
                # scalar engine's exp hides under the PE's PV matmuls
                pending = None
                for ki, (kj, mi) in enumerate(row):
                    ksl = slice(kj * 128, (kj + 1) * 128)
                    new_ests = []
                    for hh in range(2):
                        st_ps = ps_fx.tile([128, 512], F32, tag="fx")
                        nc.tensor.matmul(
                            st_ps, kT_all[:, ksl],
                            qT_all[:, hh, pr], start=True, stop=True)
                        est = at.tile([128, 512], BF16, tag="est", bufs=4)
                        nc.scalar.activation(est, st_ps, AF.Exp)
                        if mi is not None:
                            for u in range(2):
                                nc.vector.tensor_mul(
                                    est[:, u * 256:(u + 1) * 256],
                                    est[:, u * 256:(u + 1) * 256],
                                    maskt_sb[:, mi, :])
                        new_ests.append(est)
                    if pending is not None:
                        flush_pv(*pending)
                    pending = (ki, kj, new_ests)
                flush_pv(*pending)
                if pr >= 1:
                    emit_wo(pr - 1)

                # unnormalized gated output (frees the ot banks early)
                for hh in range(2):
                    nc.vector.tensor_mul(
                        otg_ring[:, 2 * hh:2 * hh + 2, slot, :],
                        ot_ps[hh].rearrange("p (u q) -> p u q", u=2),
                        sig_ring[:, 2 * hh:2 * hh + 2, (pr // 2) % 2,
                                 (pr % 2) * 256:(pr % 2) * 256 + 256])
                # normalization: the broadcast rowsums are inverted on the
                # scalar engine (all 128 partitions in parallel)
                for hh in range(2):
                    rcp = at.tile([128, 512], FP16, tag=f"rcp{hh}", bufs=2)
                    _scalar_recip(nc, rcp, rs_ps[hh])
                    og = otg_ring[:, 2 * hh:2 * hh + 2, slot, :]
                    nc.vector.tensor_mul(
                        og, og, rcp.rearrange("p (u q) -> p u q", u=2))
            emit_wo(NT // 2 - 1, final=True)

    _split_excess_waits(nc)
    return nc


_CACHE = {}
LAST_EXEC_TIME_NS = None
LAST_RESULTS = None


def _maybe_install_profile_hook():
    if not os.environ.get("BASS_TRACE"):
        return
    try:
        import sys
        import types
        import antenv
        if "antenv.axon_hooks" in sys.modules:
            return
        mod = types.ModuleType("antenv.axon_hooks")
        mod._hook = None
        mod.set_axon_ntff_profile_hook = lambda h: setattr(mod, "_hook", h)
        mod.get_axon_ntff_profile_hook = lambda: mod._hook
        sys.modules["antenv.axon_hooks"] = mod
        antenv.axon_hooks = mod
        from trn_agent_boot.trn_boot import _ntff_profile_via_ctypes
        mod.set_axon_ntff_profile_hook(
            _ntff_profile_via_ctypes("/opt/axon/libaxon_pjrt.so"))
    except Exception:
        pass


def kernel(hidden_states, cos, sin, attention_mask, Wq, Wk, Wv, Wo, Wg,
           q_norm_w, k_norm_w):
    global LAST_EXEC_TIME_NS, LAST_RESULTS
    _maybe_install_profile_hook()

    hidden_states = np.asarray(hidden_states, dtype=np.float32)
    cos = np.asarray(cos, dtype=np.float32)
    sin = np.asarray(sin, dtype=np.float32)
    mask2d = np.asarray(attention_mask, dtype=np.float32).reshape(S, S)
    Wq = np.asarray(Wq, dtype=np.float32)
    Wk = np.asarray(Wk, dtype=np.float32)
    Wv = np.asarray(Wv, dtype=np.float32)
    Wo = np.asarray(Wo, dtype=np.float32)
    Wg = np.asarray(Wg, dtype=np.float32)
    qw = np.asarray(q_norm_w, dtype=np.float32)
    kw = np.asarray(k_norm_w, dtype=np.float32)

    import ml_dtypes
    rows, mixed = _mask_plan(mask2d)
    nmix = len(mixed)
    plan_key = (tuple(tuple(r) for r in rows), nmix)
    if plan_key not in _CACHE:
        _CACHE[plan_key] = _build(rows, nmix)
    nc = _CACHE[plan_key]

    sign = np.concatenate([-np.ones(D // 2), np.ones(D // 2)]).astype(np.float32)
    qw_swap = np.concatenate([qw[D // 2:], qw[:D // 2]])
    kw_swap = np.concatenate([kw[D // 2:], kw[:D // 2]])
    maskt_np = (np.concatenate(mixed, axis=0).astype(ml_dtypes.bfloat16)
                if nmix else None)  # [nmix*128, 256]

    in_maps = []
    for c in range(8):
        b, g = divmod(c, 4)
        qs = slice(g * DQ, (g + 1) * DQ)
        ks = slice(g * D, (g + 1) * D)
        xtT = np.ascontiguousarray(hidden_states[b].T)
        m = {
            "xt16": xtT.astype(np.float16),
            "wqkv": np.ascontiguousarray(
                np.concatenate([Wq[qs], Wk[ks], Wv[ks]],
                               axis=0).T).astype(np.float16),
            "wg": np.ascontiguousarray(Wg[qs].T).astype(np.float16),
            "wo": np.ascontiguousarray(Wo[:, qs].T).astype(np.float16),
            "csw": np.ascontiguousarray(np.concatenate([
                cos[b] * qw * LAM,
                sin[b] * (sign * qw_swap) * LAM,
                cos[b] * kw,
                sin[b] * (sign * kw_swap)], axis=1)).astype(np.float16),
        }
        if nmix:
            m["maskt"] = maskt_np
        in_maps.append(m)

    res = run_bass_kernel_spmd(nc, in_maps, list(range(8)),
                               trace=bool(os.environ.get("BASS_TRACE")))
    LAST_EXEC_TIME_NS = res.exec_time_ns
    LAST_RESULTS = res

    out = np.empty((B, S, H), dtype=np.float32)
    for b in range(B):
        acc = res.results[4 * b]["y"].astype(np.float32)
        for g in range(1, 4):
            acc = acc + res.results[4 * b + g]["y"]
        out[b] = acc
    return out


# revision 24
# speedup vs baseline: 1.0068x; 1.0068x over previous
"""AFMoE attention layer on 8 NeuronCores (Trainium2, Bass/Tile).

Sharding: core c = (batch b = c//4) x (kv-head group g = c%4).
Each core computes its batch's q-heads 4g..4g+3 + kv head g end-to-end and a
partial output y_c = O_gated @ Wo[:, 512g:512(g+1)].T; the host sums the 4
group partials per batch (row-parallel Wo reduction done on host).

Structure (v3):
  phase A: qkv projection + rms-norm + rope + transposes (per 128-s-tile).
           RMS sums via scalar Square+accum, rope adds on DVE, coarse DMAs
           (the HWDGE issue cost ~0.7us/instr was the phase-A bottleneck).
  phase B: attention; gate projection interleaved per s-quarter, 2-head-
           batched score/PV/rowsum matmuls (N=512), post-exp binary mask
           multiplies on DVE, rowsums accumulated at partitions {0,32} and
           inverted directly on the scalar engine, one N=512 broadcast
           matmul per head-pair, Wo emission lagging one pair behind.
"""
import os

import numpy as np

import concourse.bass as bass
import concourse.mybir as mybir
import concourse.tile as tile
from concourse.bass_utils import run_bass_kernel_spmd
from concourse.masks import make_identity

F32 = mybir.dt.float32
F32R = mybir.dt.float32r
BF16 = mybir.dt.bfloat16
FP16 = mybir.dt.float16
AF = mybir.ActivationFunctionType
ALU = mybir.AluOpType
AX = mybir.AxisListType

B, S, H = 2, 2048, 2048
NH, NKV, D = 16, 4, 128
GROUPS = NH // NKV          # q heads per kv head = 4
QH = GROUPS                 # per-core q heads
DQ = QH * D                 # 512
EPS = 1e-5
NT = S // 128               # 16 s-tiles
HC = H // 128               # 16 h-chunks
LAM = float(D) ** -0.5
SQ = S // 4                 # 512 per s-quarter

_nsplit = [0]


def _split_excess_waits(nc, limit=1):
    """This walrus build accepts only one semaphore wait per instruction
    (fp32/fp32r matmuls included). Move excess waits onto preceding
    same-engine NoOps; engine program order keeps this correct."""
    import bass_rust
    for blk in nc.m.functions[0].blocks:
        lst = blk.instructions
        idx = 0
        while idx < len(lst):
            inst = lst[idx]
            si = inst.sync_info
            if (si is None or len(si.on_wait) <= limit
                    or type(inst).__name__ == "InstCollectiveCompute"
                    or inst.engine == mybir.EngineType.Unassigned):
                idx += 1
                continue
            waits = list(si.on_wait)
            kept, excess = waits[-limit:], waits[:-limit]
            new_insts = []
            for w in excess:
                _nsplit[0] += 1
                nop = mybir.InstNoOp(name=f"WS-{_nsplit[0]}", ins=[], outs=[])
                nop.engine = inst.engine
                nop.sync_info = bass_rust.SyncInfo(on_wait=[w], on_update=[])
                new_insts.append(nop)
            inst.sync_info = bass_rust.SyncInfo(on_wait=kept,
                                                on_update=list(si.on_update))
            lst[idx:idx] = new_insts
            idx += len(new_insts) + 1


def _scalar_recip(nc, out, in_):
    """Reciprocal on the scalar engine (bass guards this off for accuracy;
    the ~1e-3 level error is fine for this kernel's 2e-2 budget)."""
    eng = nc.scalar
    inputs = [
        eng.lower_ap(in_),
        mybir.ImmediateValue(dtype=mybir.dt.float32, value=0.0),
        mybir.ImmediateValue(dtype=mybir.dt.float32, value=1.0),
        mybir.ImmediateValue(dtype=mybir.dt.float32, value=0.0),
    ]
    outputs = [eng.lower_ap(out)]
    return eng.add_instruction(
        mybir.InstActivation(
            name=nc.get_next_instruction_name(),
            func=AF.Reciprocal,
            ins=inputs,
            outs=outputs,
        ))


def _mask_plan(mask2d):
    """Classify the additive mask in [256(q) x 128(k)] slabs (q-tile pairs).

    Returns (rows, mixed_slabs): rows[pair] = list of (kj, mixed_idx|None)
    over a contiguous kj range; mixed_slabs = transposed [128,256] np arrays
    holding BINARY (1.0 allowed / 0.0 masked) values.
    """
    nb = S // 128
    npair = nb // 2
    uniq = {}
    mixed = []
    rows = []

    def binmask(blk):
        key = blk.tobytes()
        if key not in uniq:
            uniq[key] = len(mixed)
            mixed.append(
                np.ascontiguousarray((blk.T > -1e8).astype(np.float32)))
        return uniq[key]

    def halfness(blk):
        up = (blk[128:, :] <= -1e8).all()
        low = (blk[:128, :] <= -1e8).all()
        if up and not low:
            return 0            # only the lower q-half is active
        if low and not up:
            return 1            # only the upper q-half is active
        return None

    for p in range(npair):
        qsl = slice(p * 256, (p + 1) * 256)
        entries = []
        for kj in range(nb):
            blk = mask2d[qsl, kj * 128:(kj + 1) * 128]      # [256 q, 128 k]
            if (blk <= -1e8).all():
                entries.append(None)
            elif (blk == 0.0).all():
                entries.append((kj, None, None))
            else:
                entries.append((kj, binmask(blk), halfness(blk)))
        live = [e for e in entries if e is not None]
        if not live:
            raise ValueError("fully-masked query row block unsupported")
        lo = min(e[0] for e in live)
        hi = max(e[0] for e in live)
        row = []
        for kj in range(lo, hi + 1):
            e = entries[kj]
            if e is None:
                blk = mask2d[qsl, kj * 128:(kj + 1) * 128]
                row.append((kj, binmask(blk), halfness(blk)))
            else:
                row.append(e)
        rows.append(row)
    return rows, mixed


def _build(rows, nmix):
    nc = bass.Bass()
    wqkv = nc.declare_dram_parameter("wqkv", [H, DQ + 2 * D], FP16, isOutput=False)
    xt16 = nc.declare_dram_parameter("xt16", [H, S], FP16, isOutput=False)
    wg = nc.declare_dram_parameter("wg", [H, DQ], FP16, isOutput=False)
    wo = nc.declare_dram_parameter("wo", [DQ, H], FP16, isOutput=False)
    csw = nc.declare_dram_parameter("csw", [S, 4 * D], FP16, isOutput=False)
    if nmix:
        maskt = nc.declare_dram_parameter("maskt", [nmix * 128, 256], BF16,
                                          isOutput=False)
    y = nc.declare_dram_parameter("y", [S, H], F32, isOutput=True)

    NW = DQ + 2 * D  # 768

    with tile.TileContext(nc) as tc, \
            nc.allow_low_precision(reason="fp32r matmul operands"), \
            tc.tile_pool(name="const", bufs=1) as const, \
            tc.tile_pool(name="persist", bufs=1) as pp, \
            tc.tile_pool(name="pwg", bufs=1) as pwg, \
            tc.tile_pool(name="pxtb", bufs=1) as pxtb:
        identity_f = const.tile([128, 128], F32)
        make_identity(nc, identity_f)
        identity_h = const.tile([128, 128], FP16)
        nc.vector.tensor_copy(identity_h, identity_f)
        ones_f = const.tile([128, 128], F32)
        nc.vector.memset(ones_f, 1.0)
        ones128 = const.tile([128, 128], BF16)   # rowsum-with-broadcast lhsT
        nc.vector.tensor_copy(ones128, ones_f)
        eps_t = const.tile([128, 1], F32)
        nc.vector.memset(eps_t, EPS)
        ebias_t = const.tile([128, 1], F32)
        nc.vector.memset(ebias_t, -2.0)

        qT_all = pp.tile([128, 2, NT // 2, 2, 256], FP16)     # [d, h, s]
        kT_all = pp.tile([128, S], FP16)         # [d, s]
        v_all = pp.tile([128, NT, D], BF16)      # [s-part, s-tile, d]

        wg_sb = pwg.tile([128, HC, DQ], FP16)
        # gate sigmoids, ring of 2 s-quarters
        sig_ring = pwg.tile([128, QH, 2, SQ], FP16)
        # unnormalized gated attention out, ring of 2 pairs
        otg_ring = pwg.tile([128, QH, 2, 256], FP16)
        xtb = pxtb.tile([128, HC, SQ], FP16)     # gate activations, 1 quarter

        wg2 = wg.rearrange("(c p) w -> p c w", p=128)
        xtq = xt16.rearrange("(c p) (s q) -> p c s q", p=128, s=4)

        def load_xtb(sh):
            nc.sync.dma_start(out=xtb, in_=xtq[:, :, sh, :])

        def emit_gate_with(sh, mkpg):
            """Gate projection for s-quarter sh into sig_ring[sh%2].
            Adjacent m-blocks go to different PSUM banks so consecutive
            matmuls never target the same bank."""
            for m0 in (0, 2):
                pga = mkpg()
                pgb = mkpg()
                for h in range(HC):
                    nc.tensor.matmul(
                        pga[:, :512], wg_sb[:, h, m0 * 128:(m0 + 1) * 128],
                        xtb[:, h, :],
                        start=(h == 0), stop=(h == HC - 1))
                    nc.tensor.matmul(
                        pgb[:, :512],
                        wg_sb[:, h, (m0 + 1) * 128:(m0 + 2) * 128],
                        xtb[:, h, :],
                        start=(h == 0), stop=(h == HC - 1))
                nc.scalar.activation(sig_ring[:, m0, sh % 2, :],
                                     pga[:, :512], AF.Sigmoid)
                nc.scalar.activation(sig_ring[:, m0 + 1, sh % 2, :],
                                     pgb[:, :512], AF.Sigmoid)
            if sh + 1 < 4:
                load_xtb(sh + 1)

        # ---------------- phase A: q/k/v projections + norm + rope ----------
        with tc.tile_pool(name="pwq", bufs=1) as pwq, \
                tc.tile_pool(name="pa", bufs=2) as pa, \
                tc.tile_pool(name="psa", bufs=2, space="PSUM") as psa:
            wqkv_sb = pwq.tile([128, HC, NW], FP16)
            xt4 = xt16.rearrange("(c p) (t q) -> p c t q", p=128, q=128)
            wqkv8 = wqkv.rearrange("(c f p) w -> p c f w", p=128, f=2)
            csw2 = csw.rearrange("(t f p) d -> p t f d", p=128, f=2)
            ropes = {}
            csw_t = None

            def emit_transpose(st):
                qrope, krope = ropes.pop(st)
                sl = slice(st * 128, (st + 1) * 128)
                ptq = psa.tile([128, QH, 128], FP16, tag="ptq", bufs=2)
                for h in range(QH):
                    nc.tensor.transpose(ptq[:, h, :], qrope[:, h, :],
                                        identity_h)
                ptk = psa.tile([128, 128], FP16, tag="ptk", bufs=2)
                nc.tensor.transpose(ptk, krope, identity_h)
                nc.scalar.copy(
                    qT_all[:, :, st // 2, :,
                           (st % 2) * 128:(st % 2) * 128 + 128],
                    ptq.rearrange("p (a u) q -> p a u q", a=2))
                nc.scalar.copy(kT_all[:, sl], ptk)

            for st in range(NT):
                xt_t = pa.tile([128, HC, 128], FP16, tag="xt", bufs=3)
                if st == 0:
                    nc.sync.dma_start(out=xt_t[:, 0:2, :],
                                      in_=xt4[:, 0:2, st, :])
                    nc.sync.dma_start(out=wqkv_sb[:, 0:2, :],
                                      in_=wqkv8[:, 0, :, :])
                    nc.sync.dma_start(out=xt_t[:, 2:8, :],
                                      in_=xt4[:, 2:8, st, :])
                    nc.sync.dma_start(out=xt_t[:, 8:16, :],
                                      in_=xt4[:, 8:16, st, :])
                    for c8 in range(1, 8):
                        nc.sync.dma_start(
                            out=wqkv_sb[:, 2 * c8:2 * c8 + 2, :],
                            in_=wqkv8[:, c8, :, :])
                else:
                    nc.sync.dma_start(out=xt_t[:, 0:8, :],
                                      in_=xt4[:, 0:8, st, :])
                    nc.sync.dma_start(out=xt_t[:, 8:16, :],
                                      in_=xt4[:, 8:16, st, :])
                if st % 2 == 0:
                    csw_t = pa.tile([128, 2, 4 * D], FP16, tag="csw")
                    nc.sync.dma_start(out=csw_t, in_=csw2[:, st // 2])
                if st == 6:
                    nc.sync.dma_start(out=wg_sb[:, 0:8, :],
                                      in_=wg2[:, 0:8, :])
                if st == 8:
                    nc.sync.dma_start(out=wg_sb[:, 8:16, :],
                                      in_=wg2[:, 8:16, :])
                if st == 8:
                    load_xtb(0)
                cwq_t = csw_t[:, st % 2, 0 * D:1 * D]
                swq_t = csw_t[:, st % 2, 1 * D:2 * D]
                cwk_t = csw_t[:, st % 2, 2 * D:3 * D]
                swk_t = csw_t[:, st % 2, 3 * D:4 * D]

                pqkv = psa.tile([128, NW], F32, tag="pqkv", bufs=2)
                for h in range(HC):
                    nc.tensor.matmul(pqkv[:, :DQ], xt_t[:, h, :],
                                     wqkv_sb[:, h, :DQ],
                                     start=(h == 0), stop=(h == HC - 1))
                    nc.tensor.matmul(pqkv[:, DQ:], xt_t[:, h, :],
                                     wqkv_sb[:, h, DQ:],
                                     start=(h == 0), stop=(h == HC - 1))
                q_raw = pa.tile([128, DQ], F32, tag="qraw")
                nc.scalar.copy(q_raw, pqkv[:, :DQ])
                k_raw = pa.tile([128, D], F32, tag="kraw")
                nc.scalar.copy(k_raw, pqkv[:, DQ:DQ + D])
                nc.scalar.copy(v_all[:, st, :], pqkv[:, DQ + D:])

                # rms-norm sums on the scalar engine (Square + accumulate)
                sq = pa.tile([128, D], F32, tag="sq")
                ssq = pa.tile([128, QH + 1], F32, tag="ssq")
                for h in range(QH):
                    nc.scalar.activation(sq, q_raw[:, h * D:(h + 1) * D],
                                         AF.Square,
                                         accum_out=ssq[:, h:h + 1])
                nc.scalar.activation(sq, k_raw, AF.Square,
                                     accum_out=ssq[:, QH:QH + 1])
                rtq = pa.tile([128, QH + 1], F32, tag="rtq")
                nc.scalar.activation(rtq, ssq, AF.Sqrt, bias=eps_t,
                                     scale=1.0 / D)
                rq = pa.tile([128, QH + 1], F32, tag="rq")
                nc.vector.reciprocal(rq, rtq)

                # rope swaps (half-rotations) of the raw values
                r_q = pa.tile([128, QH, D], F32, tag="rqrot")
                qv = q_raw.rearrange("p (h s d) -> p h s d", h=QH, s=2)
                rv = r_q.rearrange("p h (s d) -> p h s d", s=2)
                nc.gpsimd.tensor_copy(out=rv[:, :, 0, :], in_=qv[:, :, 1, :])
                nc.gpsimd.tensor_copy(out=rv[:, :, 1, :], in_=qv[:, :, 0, :])
                r_k = pa.tile([128, D], F32, tag="rkrot")
                nc.gpsimd.tensor_copy(out=r_k[:, :64], in_=k_raw[:, 64:])
                nc.gpsimd.tensor_copy(out=r_k[:, 64:], in_=k_raw[:, :64])

                qrope = pa.tile([128, QH, D], FP16, tag="qrope", bufs=3)
                qh = q_raw.rearrange("p (h d) -> p h d", d=D)
                for h in range(QH):
                    nc.vector.scalar_tensor_tensor(
                        qrope[:, h, :], qh[:, h, :], rq[:, h:h + 1], cwq_t,
                        op0=ALU.mult, op1=ALU.mult)
                    nc.vector.scalar_tensor_tensor(
                        r_q[:, h, :], r_q[:, h, :], rq[:, h:h + 1], swq_t,
                        op0=ALU.mult, op1=ALU.mult)
                nc.vector.tensor_add(qrope, qrope, r_q)

                krope = pa.tile([128, D], FP16, tag="krope", bufs=3)
                nc.vector.scalar_tensor_tensor(krope, k_raw,
                                               rq[:, QH:QH + 1], cwk_t,
                                               op0=ALU.mult, op1=ALU.mult)
                nc.vector.scalar_tensor_tensor(r_k, r_k, rq[:, QH:QH + 1],
                                               swk_t,
                                               op0=ALU.mult, op1=ALU.mult)
                nc.vector.tensor_add(krope, krope, r_k)

                ropes[st] = (qrope, krope)
                if st >= 2:
                    emit_transpose(st - 2)
            emit_gate_with(0, lambda: psa.tile(
                [128, NW], F32, tag="pqkv", bufs=2, name="pg0"))
            emit_transpose(NT - 2)
            emit_transpose(NT - 1)

        # ---------------- phase B: attention + gating + Wo ------------------
        with tc.tile_pool(name="atw", bufs=1) as atw, \
                tc.tile_pool(name="at", bufs=2) as at, \
                tc.tile_pool(name="po", bufs=2) as po, \
                tc.tile_pool(name="ps_fx", bufs=2, space="PSUM") as ps_fx, \
                tc.tile_pool(name="ps_ot", bufs=2, space="PSUM") as ps_ot, \
                tc.tile_pool(name="ps_rs", bufs=2, space="PSUM") as ps_rs, \
                tc.tile_pool(name="ps_y", bufs=2, space="PSUM") as ps_y:
            wo_sb = atw.tile([128, QH, H], FP16)
            wo2 = wo.rearrange("(c p) w -> p c w", p=128)
            nc.sync.dma_start(out=wo_sb[:, 0:2, :], in_=wo2[:, 0:2, :])
            nc.sync.dma_start(out=wo_sb[:, 2:4, :], in_=wo2[:, 2:4, :])
            if nmix:
                maskt_sb = atw.tile([128, nmix, 256], BF16)
                mt = maskt.rearrange("(m p) q -> p m q", p=128)
                nc.sync.dma_start(out=maskt_sb, in_=mt)


            def emit_gate(sh):
                emit_gate_with(sh, lambda: ps_y.tile(
                    [128, 512], F32, tag="py", name="pg"))

            def emit_wo(pr, final=False):
                slot = pr % 2
                for sti in range(2):
                    st = 2 * pr + sti
                    y_sb = po.tile([128, H], F32, tag="ysb")
                    for n4 in (0, 2):
                        pya = ps_y.tile([128, 512], F32, tag="py")
                        pyb = ps_y.tile([128, 512], F32, tag="py")
                        for dc in range(QH):
                            lhs = otg_ring[:, dc, slot,
                                           sti * 128:(sti + 1) * 128]
                            nc.tensor.matmul(
                                pya, lhs,
                                wo_sb[:, dc, n4 * 512:(n4 + 1) * 512],
                                start=(dc == 0), stop=(dc == QH - 1))
                            nc.tensor.matmul(
                                pyb, lhs,
                                wo_sb[:, dc, (n4 + 1) * 512:(n4 + 2) * 512],
                                start=(dc == 0), stop=(dc == QH - 1))
                        nc.vector.tensor_copy(
                            y_sb[:, n4 * 512:(n4 + 1) * 512], pya)
                        if final:
                            nc.sync.dma_start(
                                out=y[st * 128:(st + 1) * 128,
                                      n4 * 512:(n4 + 1) * 512],
                                in_=y_sb[:, n4 * 512:(n4 + 1) * 512])
                        nc.vector.tensor_copy(
                            y_sb[:, (n4 + 1) * 512:(n4 + 2) * 512], pyb)
                        if final:
                            nc.sync.dma_start(
                                out=y[st * 128:(st + 1) * 128,
                                      (n4 + 1) * 512:(n4 + 2) * 512],
                                in_=y_sb[:, (n4 + 1) * 512:(n4 + 2) * 512])
                    if not final:
                        nc.sync.dma_start(
                            out=y[st * 128:(st + 1) * 128, :], in_=y_sb)

            for pr in range(NT // 2):
                if pr % 2 == 0 and pr > 0:
                    emit_gate(pr // 2)
                row = rows[pr]
                qsl = slice(pr * 256, (pr + 1) * 256)
                slot = pr % 2
                ot_ps = []
                rs_ps = []
                for _hh in range(2):
                    ot_tile = ps_ot.tile([128, 512], F32, tag="ot")
                    ot_ps.append(ot_tile)
                    rs_tile = ps_rs.tile([128, 512], F32, tag="rs")
                    rs_ps.append(rs_tile)
                nk = len(row)

                def flush_pv(ki, kj, half, ests):
                    for hh in range(2):
                        est = ests[hh]
                        if half is None:
                            oslc = ot_ps[hh]
                            rslc = rs_ps[hh]
                            eslc = est
                        else:
                            oslc = ot_ps[hh].rearrange(
                                "p (u q) -> p u q", u=2)[
                                :, :, half * 128:half * 128 + 128]
                            rslc = rs_ps[hh].rearrange(
                                "p (u q) -> p u q", u=2)[
                                :, :, half * 128:half * 128 + 128]
                            eslc = est[:, 0:256]
                        nc.tensor.matmul(
                            oslc, v_all[:, kj, :], eslc,
                            start=(ki == 0), stop=(ki == nk - 1))
                        nc.tensor.matmul(
                            rslc, ones128, eslc,
                            start=(ki == 0), stop=(ki == nk - 1))

                # scores+exp run one k-block ahead of PV/rowsum so the
                # scalar engine's exp hides under the PE's PV matmuls
                pending = None
                for ki, (kj, mi, half) in enumerate(row):
                    ksl = slice(kj * 128, (kj + 1) * 128)
                    new_ests = []
                    for hh in range(2):
                        st_ps = ps_fx.tile([128, 512], F32, tag="fx")
                        est = at.tile([128, 512], BF16, tag="est", bufs=4)
                        if half is None:
                            nc.tensor.matmul(
                                st_ps, kT_all[:, ksl],
                                qT_all[:, hh, pr], start=True, stop=True)
                            nc.scalar.activation(est, st_ps, AF.Exp)
                            if mi is not None:
                                for u in range(2):
                                    nc.vector.tensor_mul(
                                        est[:, u * 256:(u + 1) * 256],
                                        est[:, u * 256:(u + 1) * 256],
                                        maskt_sb[:, mi, :])
                        else:
                            hsl = slice(half * 128, half * 128 + 128)
                            nc.tensor.matmul(
                                st_ps[:, 0:256], kT_all[:, ksl],
                                qT_all[:, hh, pr, :, hsl],
                                start=True, stop=True)
                            nc.scalar.activation(est[:, 0:256],
                                                 st_ps[:, 0:256], AF.Exp)
                            for u in range(2):
                                nc.vector.tensor_mul(
                                    est[:, u * 128:(u + 1) * 128],
                                    est[:, u * 128:(u + 1) * 128],
                                    maskt_sb[:, mi, hsl])
                        new_ests.append(est)
                    if pending is not None:
                        flush_pv(*pending)
                    pending = (ki, kj, half, new_ests)
                flush_pv(*pending)
                if pr >= 1:
                    emit_wo(pr - 1)

                # unnormalized gated output (frees the ot banks early)
                for hh in range(2):
                    nc.vector.tensor_mul(
                        otg_ring[:, 2 * hh:2 * hh + 2, slot, :],
                        ot_ps[hh].rearrange("p (u q) -> p u q", u=2),
                        sig_ring[:, 2 * hh:2 * hh + 2, (pr // 2) % 2,
                                 (pr % 2) * 256:(pr % 2) * 256 + 256])
                # normalization: the broadcast rowsums are inverted on the
                # scalar engine (all 128 partitions in parallel)
                for hh in range(2):
                    rcp = at.tile([128, 512], FP16, tag=f"rcp{hh}", bufs=2)
                    _scalar_recip(nc, rcp, rs_ps[hh])
                    og = otg_ring[:, 2 * hh:2 * hh + 2, slot, :]
                    nc.vector.tensor_mul(
                        og, og, rcp.rearrange("p (u q) -> p u q", u=2))
            emit_wo(NT // 2 - 1, final=True)

    _split_excess_waits(nc)
    return nc


_CACHE = {}
LAST_EXEC_TIME_NS = None
LAST_RESULTS = None


def _maybe_install_profile_hook():
    if not os.environ.get("BASS_TRACE"):
        return
    try:
        import sys
        import types
        import antenv
        if "antenv.axon_hooks" in sys.modules:
            return
        mod = types.ModuleType("antenv.axon_hooks")
        mod._hook = None
        mod.set_axon_ntff_profile_hook = lambda h: setattr(mod, "_hook", h)
        mod.get_axon_ntff_profile_hook = lambda: mod._hook
        sys.modules["antenv.axon_hooks"] = mod
        antenv.axon_hooks = mod
        from trn_agent_boot.trn_boot import _ntff_profile_via_ctypes
        mod.set_axon_ntff_profile_hook(
            _ntff_profile_via_ctypes("/opt/axon/libaxon_pjrt.so"))
    except Exception:
        pass


def kernel(hidden_states, cos, sin, attention_mask, Wq, Wk, Wv, Wo, Wg,
           q_norm_w, k_norm_w):
    global LAST_EXEC_TIME_NS, LAST_RESULTS
    _maybe_install_profile_hook()

    hidden_states = np.asarray(hidden_states, dtype=np.float32)
    cos = np.asarray(cos, dtype=np.float32)
    sin = np.asarray(sin, dtype=np.float32)
    mask2d = np.asarray(attention_mask, dtype=np.float32).reshape(S, S)
    Wq = np.asarray(Wq, dtype=np.float32)
    Wk = np.asarray(Wk, dtype=np.float32)
    Wv = np.asarray(Wv, dtype=np.float32)
    Wo = np.asarray(Wo, dtype=np.float32)
    Wg = np.asarray(Wg, dtype=np.float32)
    qw = np.asarray(q_norm_w, dtype=np.float32)
    kw = np.asarray(k_norm_w, dtype=np.float32)

    import ml_dtypes
    rows, mixed = _mask_plan(mask2d)
    nmix = len(mixed)
    plan_key = (tuple(tuple(r) for r in rows), nmix)
    if plan_key not in _CACHE:
        _CACHE[plan_key] = _build(rows, nmix)
    nc = _CACHE[plan_key]

    sign = np.concatenate([-np.ones(D // 2), np.ones(D // 2)]).astype(np.float32)
    qw_swap = np.concatenate([qw[D // 2:], qw[:D // 2]])
    kw_swap = np.concatenate([kw[D // 2:], kw[:D // 2]])
    maskt_np = (np.concatenate(mixed, axis=0).astype(ml_dtypes.bfloat16)
                if nmix else None)  # [nmix*128, 256]

    in_maps = []
    for c in range(8):
        b, g = divmod(c, 4)
        qs = slice(g * DQ, (g + 1) * DQ)
        ks = slice(g * D, (g + 1) * D)
        xtT = np.ascontiguousarray(hidden_states[b].T)
        m = {
            "xt16": xtT.astype(np.float16),
            "wqkv": np.ascontiguousarray(
                np.concatenate([Wq[qs], Wk[ks], Wv[ks]],
                               axis=0).T).astype(np.float16),
            "wg": np.ascontiguousarray(Wg[qs].T).astype(np.float16),
            "wo": np.ascontiguousarray(Wo[:, qs].T).astype(np.float16),
            "csw": np.ascontiguousarray(np.concatenate([
                cos[b] * qw * LAM,
                sin[b] * (sign * qw_swap) * LAM,
                cos[b] * kw,
                sin[b] * (sign * kw_swap)], axis=1)).astype(np.float16),
        }
        if nmix:
            m["maskt"] = maskt_np
        in_maps.append(m)

    res = run_bass_kernel_spmd(nc, in_maps, list(range(8)),
                               trace=bool(os.environ.get("BASS_TRACE")))
    LAST_EXEC_TIME_NS = res.exec_time_ns
    LAST_RESULTS = res

    out = np.empty((B, S, H), dtype=np.float32)
    for b in range(B):
        acc = res.results[4 * b]["y"].astype(np.float32)
        for g in range(1, 4):
            acc = acc + res.results[4 * b + g]["y"]
        out[b] = acc
    return out


# revision 25
# speedup vs baseline: 1.0164x; 1.0095x over previous
"""AFMoE attention layer on 8 NeuronCores (Trainium2, Bass/Tile).

Sharding: core c = (batch b = c//4) x (kv-head group g = c%4).
Each core computes its batch's q-heads 4g..4g+3 + kv head g end-to-end and a
partial output y_c = O_gated @ Wo[:, 512g:512(g+1)].T; the host sums the 4
group partials per batch (row-parallel Wo reduction done on host).

Structure (v3):
  phase A: qkv projection + rms-norm + rope + transposes (per 128-s-tile).
           RMS sums via scalar Square+accum, rope adds on DVE, coarse DMAs
           (the HWDGE issue cost ~0.7us/instr was the phase-A bottleneck).
  phase B: attention; gate projection interleaved per s-quarter, 2-head-
           batched score/PV/rowsum matmuls (N=512), post-exp binary mask
           multiplies on DVE, rowsums accumulated at partitions {0,32} and
           inverted directly on the scalar engine, one N=512 broadcast
           matmul per head-pair, Wo emission lagging one pair behind.
"""
import os

import numpy as np

import concourse.bass as bass
import concourse.mybir as mybir
import concourse.tile as tile
from concourse.bass_utils import run_bass_kernel_spmd
from concourse.masks import make_identity

F32 = mybir.dt.float32
F32R = mybir.dt.float32r
BF16 = mybir.dt.bfloat16
FP16 = mybir.dt.float16
AF = mybir.ActivationFunctionType
ALU = mybir.AluOpType
AX = mybir.AxisListType

B, S, H = 2, 2048, 2048
NH, NKV, D = 16, 4, 128
GROUPS = NH // NKV          # q heads per kv head = 4
QH = GROUPS                 # per-core q heads
DQ = QH * D                 # 512
EPS = 1e-5
NT = S // 128               # 16 s-tiles
HC = H // 128               # 16 h-chunks
LAM = float(D) ** -0.5
SQ = S // 4                 # 512 per s-quarter

_nsplit = [0]


def _split_excess_waits(nc, limit=1):
    """This walrus build accepts only one semaphore wait per instruction
    (fp32/fp32r matmuls included). Move excess waits onto preceding
    same-engine NoOps; engine program order keeps this correct."""
    import bass_rust
    for blk in nc.m.functions[0].blocks:
        lst = blk.instructions
        idx = 0
        while idx < len(lst):
            inst = lst[idx]
            si = inst.sync_info
            if (si is None or len(si.on_wait) <= limit
                    or type(inst).__name__ == "InstCollectiveCompute"
                    or inst.engine == mybir.EngineType.Unassigned):
                idx += 1
                continue
            waits = list(si.on_wait)
            kept, excess = waits[-limit:], waits[:-limit]
            new_insts = []
            for w in excess:
                _nsplit[0] += 1
                nop = mybir.InstNoOp(name=f"WS-{_nsplit[0]}", ins=[], outs=[])
                nop.engine = inst.engine
                nop.sync_info = bass_rust.SyncInfo(on_wait=[w], on_update=[])
                new_insts.append(nop)
            inst.sync_info = bass_rust.SyncInfo(on_wait=kept,
                                                on_update=list(si.on_update))
            lst[idx:idx] = new_insts
            idx += len(new_insts) + 1


def _scalar_recip(nc, out, in_):
    """Reciprocal on the scalar engine (bass guards this off for accuracy;
    the ~1e-3 level error is fine for this kernel's 2e-2 budget)."""
    eng = nc.scalar
    inputs = [
        eng.lower_ap(in_),
        mybir.ImmediateValue(dtype=mybir.dt.float32, value=0.0),
        mybir.ImmediateValue(dtype=mybir.dt.float32, value=1.0),
        mybir.ImmediateValue(dtype=mybir.dt.float32, value=0.0),
    ]
    outputs = [eng.lower_ap(out)]
    return eng.add_instruction(
        mybir.InstActivation(
            name=nc.get_next_instruction_name(),
            func=AF.Reciprocal,
            ins=inputs,
            outs=outputs,
        ))


def _mask_plan(mask2d):
    """Classify the additive mask in [256(q) x 128(k)] slabs (q-tile pairs).

    Returns (rows, mixed_slabs): rows[pair] = list of (kj, mixed_idx|None)
    over a contiguous kj range; mixed_slabs = transposed [128,256] np arrays
    holding BINARY (1.0 allowed / 0.0 masked) values.
    """
    nb = S // 128
    npair = nb // 2
    uniq = {}
    mixed = []
    rows = []

    def binmask(blk):
        key = blk.tobytes()
        if key not in uniq:
            uniq[key] = len(mixed)
            mixed.append(
                np.ascontiguousarray((blk.T > -1e8).astype(np.float32)))
        return uniq[key]

    def halfness(blk):
        up = (blk[128:, :] <= -1e8).all()
        low = (blk[:128, :] <= -1e8).all()
        if up and not low:
            return 0            # only the lower q-half is active
        if low and not up:
            return 1            # only the upper q-half is active
        return None

    for p in range(npair):
        qsl = slice(p * 256, (p + 1) * 256)
        entries = []
        for kj in range(nb):
            blk = mask2d[qsl, kj * 128:(kj + 1) * 128]      # [256 q, 128 k]
            if (blk <= -1e8).all():
                entries.append(None)
            elif (blk == 0.0).all():
                entries.append((kj, None, None))
            else:
                entries.append((kj, binmask(blk), halfness(blk)))
        live = [e for e in entries if e is not None]
        if not live:
            raise ValueError("fully-masked query row block unsupported")
        lo = min(e[0] for e in live)
        hi = max(e[0] for e in live)
        row = []
        for kj in range(lo, hi + 1):
            e = entries[kj]
            if e is None:
                blk = mask2d[qsl, kj * 128:(kj + 1) * 128]
                row.append((kj, binmask(blk), halfness(blk)))
            else:
                row.append(e)
        rows.append(row)
    return rows, mixed


def _build(rows, nmix):
    nc = bass.Bass()
    wqkv = nc.declare_dram_parameter("wqkv", [H, DQ + 2 * D], FP16, isOutput=False)
    xt16 = nc.declare_dram_parameter("xt16", [H, S], FP16, isOutput=False)
    wg = nc.declare_dram_parameter("wg", [H, DQ], FP16, isOutput=False)
    wo = nc.declare_dram_parameter("wo", [DQ, H], FP16, isOutput=False)
    csw = nc.declare_dram_parameter("csw", [S, 4 * D], FP16, isOutput=False)
    if nmix:
        maskt = nc.declare_dram_parameter("maskt", [nmix * 128, 256], BF16,
                                          isOutput=False)
    y = nc.declare_dram_parameter("y", [S, H], F32, isOutput=True)

    NW = DQ + 2 * D  # 768

    with tile.TileContext(nc) as tc, \
            nc.allow_low_precision(reason="fp32r matmul operands"), \
            tc.tile_pool(name="const", bufs=1) as const, \
            tc.tile_pool(name="persist", bufs=1) as pp, \
            tc.tile_pool(name="pwg", bufs=1) as pwg, \
            tc.tile_pool(name="pxtb", bufs=1) as pxtb:
        identity_f = const.tile([128, 128], F32)
        make_identity(nc, identity_f)
        identity_h = const.tile([128, 128], FP16)
        nc.vector.tensor_copy(identity_h, identity_f)
        ones_f = const.tile([128, 128], F32)
        nc.vector.memset(ones_f, 1.0)
        ones128 = const.tile([128, 128], BF16)   # rowsum-with-broadcast lhsT
        nc.vector.tensor_copy(ones128, ones_f)
        eps_t = const.tile([128, 1], F32)
        nc.vector.memset(eps_t, EPS)
        ebias_t = const.tile([128, 1], F32)
        nc.vector.memset(ebias_t, -2.0)

        qT_all = pp.tile([128, 2, NT // 2, 2, 256], FP16)     # [d, h, s]
        kT_all = pp.tile([128, S], FP16)         # [d, s]
        v_all = pp.tile([128, NT, D], BF16)      # [s-part, s-tile, d]

        wg_sb = pwg.tile([128, HC, DQ], FP16)
        # gate sigmoids, ring of 2 s-quarters
        sig_ring = pwg.tile([128, QH, 2, SQ], FP16)
        # unnormalized gated attention out, ring of 2 pairs
        otg_ring = pwg.tile([128, QH, 2, 256], FP16)
        xtb = pxtb.tile([128, HC, SQ], FP16)     # gate activations, 1 quarter

        wg2 = wg.rearrange("(c p) w -> p c w", p=128)
        xtq = xt16.rearrange("(c p) (s q) -> p c s q", p=128, s=4)

        def load_xtb(sh):
            nc.sync.dma_start(out=xtb, in_=xtq[:, :, sh, :])

        def emit_gate_with(sh, mkpg, halves=(0, 2)):
            """Gate projection for s-quarter sh into sig_ring[sh%2].
            Adjacent m-blocks go to different PSUM banks so consecutive
            matmuls never target the same bank."""
            for m0 in halves:
                pga = mkpg()
                pgb = mkpg()
                for h in range(HC):
                    nc.tensor.matmul(
                        pga[:, :512], wg_sb[:, h, m0 * 128:(m0 + 1) * 128],
                        xtb[:, h, :],
                        start=(h == 0), stop=(h == HC - 1))
                    nc.tensor.matmul(
                        pgb[:, :512],
                        wg_sb[:, h, (m0 + 1) * 128:(m0 + 2) * 128],
                        xtb[:, h, :],
                        start=(h == 0), stop=(h == HC - 1))
                nc.scalar.activation(sig_ring[:, m0, sh % 2, :],
                                     pga[:, :512], AF.Sigmoid)
                nc.scalar.activation(sig_ring[:, m0 + 1, sh % 2, :],
                                     pgb[:, :512], AF.Sigmoid)
            if sh + 1 < 4 and halves[-1] == 2:
                load_xtb(sh + 1)

        # ---------------- phase A: q/k/v projections + norm + rope ----------
        with tc.tile_pool(name="pwq", bufs=1) as pwq, \
                tc.tile_pool(name="pa", bufs=2) as pa, \
                tc.tile_pool(name="psa", bufs=2, space="PSUM") as psa:
            wqkv_sb = pwq.tile([128, HC, NW], FP16)
            xt4 = xt16.rearrange("(c p) (t q) -> p c t q", p=128, q=128)
            wqkv8 = wqkv.rearrange("(c f p) w -> p c f w", p=128, f=2)
            csw2 = csw.rearrange("(t f p) d -> p t f d", p=128, f=2)
            ropes = {}
            csw_t = None

            def emit_transpose(st):
                qrope, krope = ropes.pop(st)
                sl = slice(st * 128, (st + 1) * 128)
                ptq = psa.tile([128, QH, 128], FP16, tag="ptq", bufs=2)
                for h in range(QH):
                    nc.tensor.transpose(ptq[:, h, :], qrope[:, h, :],
                                        identity_h)
                ptk = psa.tile([128, 128], FP16, tag="ptk", bufs=2)
                nc.tensor.transpose(ptk, krope, identity_h)
                nc.scalar.copy(
                    qT_all[:, :, st // 2, :,
                           (st % 2) * 128:(st % 2) * 128 + 128],
                    ptq.rearrange("p (a u) q -> p a u q", a=2))
                nc.scalar.copy(kT_all[:, sl], ptk)

            for st in range(NT):
                xt_t = pa.tile([128, HC, 128], FP16, tag="xt", bufs=3)
                if st == 0:
                    nc.sync.dma_start(out=xt_t[:, 0:2, :],
                                      in_=xt4[:, 0:2, st, :])
                    nc.sync.dma_start(out=wqkv_sb[:, 0:2, :],
                                      in_=wqkv8[:, 0, :, :])
                    nc.sync.dma_start(out=xt_t[:, 2:8, :],
                                      in_=xt4[:, 2:8, st, :])
                    nc.sync.dma_start(out=xt_t[:, 8:16, :],
                                      in_=xt4[:, 8:16, st, :])
                    for c8 in range(1, 8):
                        nc.sync.dma_start(
                            out=wqkv_sb[:, 2 * c8:2 * c8 + 2, :],
                            in_=wqkv8[:, c8, :, :])
                else:
                    nc.sync.dma_start(out=xt_t[:, 0:8, :],
                                      in_=xt4[:, 0:8, st, :])
                    nc.sync.dma_start(out=xt_t[:, 8:16, :],
                                      in_=xt4[:, 8:16, st, :])
                if st % 2 == 0:
                    csw_t = pa.tile([128, 2, 4 * D], FP16, tag="csw")
                    nc.sync.dma_start(out=csw_t, in_=csw2[:, st // 2])
                if st == 6:
                    nc.sync.dma_start(out=wg_sb[:, 0:8, :],
                                      in_=wg2[:, 0:8, :])
                if st == 8:
                    nc.sync.dma_start(out=wg_sb[:, 8:16, :],
                                      in_=wg2[:, 8:16, :])
                if st == 8:
                    load_xtb(0)
                cwq_t = csw_t[:, st % 2, 0 * D:1 * D]
                swq_t = csw_t[:, st % 2, 1 * D:2 * D]
                cwk_t = csw_t[:, st % 2, 2 * D:3 * D]
                swk_t = csw_t[:, st % 2, 3 * D:4 * D]

                pqkv = psa.tile([128, NW], F32, tag="pqkv", bufs=2)
                for h in range(HC):
                    nc.tensor.matmul(pqkv[:, :DQ], xt_t[:, h, :],
                                     wqkv_sb[:, h, :DQ],
                                     start=(h == 0), stop=(h == HC - 1))
                    nc.tensor.matmul(pqkv[:, DQ:], xt_t[:, h, :],
                                     wqkv_sb[:, h, DQ:],
                                     start=(h == 0), stop=(h == HC - 1))
                q_raw = pa.tile([128, DQ], F32, tag="qraw")
                nc.scalar.copy(q_raw, pqkv[:, :DQ])
                k_raw = pa.tile([128, D], F32, tag="kraw")
                nc.scalar.copy(k_raw, pqkv[:, DQ:DQ + D])
                nc.scalar.copy(v_all[:, st, :], pqkv[:, DQ + D:])

                # rms-norm sums on the scalar engine (Square + accumulate)
                sq = pa.tile([128, D], F32, tag="sq")
                ssq = pa.tile([128, QH + 1], F32, tag="ssq")
                for h in range(QH):
                    nc.scalar.activation(sq, q_raw[:, h * D:(h + 1) * D],
                                         AF.Square,
                                         accum_out=ssq[:, h:h + 1])
                nc.scalar.activation(sq, k_raw, AF.Square,
                                     accum_out=ssq[:, QH:QH + 1])
                rtq = pa.tile([128, QH + 1], F32, tag="rtq")
                nc.scalar.activation(rtq, ssq, AF.Sqrt, bias=eps_t,
                                     scale=1.0 / D)
                rq = pa.tile([128, QH + 1], F32, tag="rq")
                nc.vector.reciprocal(rq, rtq)

                # rope swaps (half-rotations) of the raw values
                r_q = pa.tile([128, QH, D], F32, tag="rqrot")
                qv = q_raw.rearrange("p (h s d) -> p h s d", h=QH, s=2)
                rv = r_q.rearrange("p h (s d) -> p h s d", s=2)
                nc.gpsimd.tensor_copy(out=rv[:, :, 0, :], in_=qv[:, :, 1, :])
                nc.gpsimd.tensor_copy(out=rv[:, :, 1, :], in_=qv[:, :, 0, :])
                r_k = pa.tile([128, D], F32, tag="rkrot")
                nc.gpsimd.tensor_copy(out=r_k[:, :64], in_=k_raw[:, 64:])
                nc.gpsimd.tensor_copy(out=r_k[:, 64:], in_=k_raw[:, :64])

                qrope = pa.tile([128, QH, D], FP16, tag="qrope", bufs=3)
                qh = q_raw.rearrange("p (h d) -> p h d", d=D)
                for h in range(QH):
                    nc.vector.scalar_tensor_tensor(
                        qrope[:, h, :], qh[:, h, :], rq[:, h:h + 1], cwq_t,
                        op0=ALU.mult, op1=ALU.mult)
                    nc.vector.scalar_tensor_tensor(
                        r_q[:, h, :], r_q[:, h, :], rq[:, h:h + 1], swq_t,
                        op0=ALU.mult, op1=ALU.mult)
                nc.vector.tensor_add(qrope, qrope, r_q)

                krope = pa.tile([128, D], FP16, tag="krope", bufs=3)
                nc.vector.scalar_tensor_tensor(krope, k_raw,
                                               rq[:, QH:QH + 1], cwk_t,
                                               op0=ALU.mult, op1=ALU.mult)
                nc.vector.scalar_tensor_tensor(r_k, r_k, rq[:, QH:QH + 1],
                                               swk_t,
                                               op0=ALU.mult, op1=ALU.mult)
                nc.vector.tensor_add(krope, krope, r_k)

                ropes[st] = (qrope, krope)
                if st >= 2:
                    emit_transpose(st - 2)
            emit_gate_with(0, lambda: psa.tile(
                [128, NW], F32, tag="pqkv", bufs=2, name="pg0"),
                halves=(0,))
            emit_transpose(NT - 2)
            emit_gate_with(0, lambda: psa.tile(
                [128, NW], F32, tag="pqkv", bufs=2, name="pg0b"),
                halves=(2,))
            emit_transpose(NT - 1)

        # ---------------- phase B: attention + gating + Wo ------------------
        with tc.tile_pool(name="atw", bufs=1) as atw, \
                tc.tile_pool(name="at", bufs=2) as at, \
                tc.tile_pool(name="po", bufs=2) as po, \
                tc.tile_pool(name="ps_fx", bufs=2, space="PSUM") as ps_fx, \
                tc.tile_pool(name="ps_ot", bufs=2, space="PSUM") as ps_ot, \
                tc.tile_pool(name="ps_rs", bufs=2, space="PSUM") as ps_rs, \
                tc.tile_pool(name="ps_y", bufs=2, space="PSUM") as ps_y:
            wo_sb = atw.tile([128, QH, H], FP16)
            wo2 = wo.rearrange("(c p) w -> p c w", p=128)
            nc.sync.dma_start(out=wo_sb[:, 0:2, :], in_=wo2[:, 0:2, :])
            nc.sync.dma_start(out=wo_sb[:, 2:4, :], in_=wo2[:, 2:4, :])
            if nmix:
                maskt_sb = atw.tile([128, nmix, 256], BF16)
                mt = maskt.rearrange("(m p) q -> p m q", p=128)
                nc.sync.dma_start(out=maskt_sb, in_=mt)


            def emit_gate(sh):
                emit_gate_with(sh, lambda: ps_y.tile(
                    [128, 512], F32, tag="py", name="pg"))

            def emit_wo(pr, final=False):
                slot = pr % 2
                for sti in range(2):
                    st = 2 * pr + sti
                    y_sb = po.tile([128, H], F32, tag="ysb")
                    for n4 in (0, 2):
                        pya = ps_y.tile([128, 512], F32, tag="py")
                        pyb = ps_y.tile([128, 512], F32, tag="py")
                        for dc in range(QH):
                            lhs = otg_ring[:, dc, slot,
                                           sti * 128:(sti + 1) * 128]
                            nc.tensor.matmul(
                                pya, lhs,
                                wo_sb[:, dc, n4 * 512:(n4 + 1) * 512],
                                start=(dc == 0), stop=(dc == QH - 1))
                            nc.tensor.matmul(
                                pyb, lhs,
                                wo_sb[:, dc, (n4 + 1) * 512:(n4 + 2) * 512],
                                start=(dc == 0), stop=(dc == QH - 1))
                        nc.vector.tensor_copy(
                            y_sb[:, n4 * 512:(n4 + 1) * 512], pya)
                        if final:
                            nc.sync.dma_start(
                                out=y[st * 128:(st + 1) * 128,
                                      n4 * 512:(n4 + 1) * 512],
                                in_=y_sb[:, n4 * 512:(n4 + 1) * 512])
                        nc.vector.tensor_copy(
                            y_sb[:, (n4 + 1) * 512:(n4 + 2) * 512], pyb)
                        if final:
                            nc.sync.dma_start(
                                out=y[st * 128:(st + 1) * 128,
                                      (n4 + 1) * 512:(n4 + 2) * 512],
                                in_=y_sb[:, (n4 + 1) * 512:(n4 + 2) * 512])
                    if not final:
                        nc.sync.dma_start(
                            out=y[st * 128:(st + 1) * 128, :], in_=y_sb)

            for pr in range(NT // 2):
                if pr % 2 == 0 and pr > 0:
                    emit_gate(pr // 2)
                row = rows[pr]
                qsl = slice(pr * 256, (pr + 1) * 256)
                slot = pr % 2
                ot_ps = []
                rs_ps = []
                for _hh in range(2):
                    ot_tile = ps_ot.tile([128, 512], F32, tag="ot")
                    ot_ps.append(ot_tile)
                    rs_tile = ps_rs.tile([128, 512], F32, tag="rs")
                    rs_ps.append(rs_tile)
                nk = len(row)

                def flush_pv(ki, kj, half, ests):
                    for hh in range(2):
                        est = ests[hh]
                        if half is None:
                            oslc = ot_ps[hh]
                            rslc = rs_ps[hh]
                            eslc = est
                        else:
                            oslc = ot_ps[hh].rearrange(
                                "p (u q) -> p u q", u=2)[
                                :, :, half * 128:half * 128 + 128]
                            rslc = rs_ps[hh].rearrange(
                                "p (u q) -> p u q", u=2)[
                                :, :, half * 128:half * 128 + 128]
                            eslc = est[:, 0:256]
                        nc.tensor.matmul(
                            oslc, v_all[:, kj, :], eslc,
                            start=(ki == 0), stop=(ki == nk - 1))
                        nc.tensor.matmul(
                            rslc, ones128, eslc,
                            start=(ki == 0), stop=(ki == nk - 1))

                # scores+exp run one k-block ahead of PV/rowsum so the
                # scalar engine's exp hides under the PE's PV matmuls
                pending = None
                for ki, (kj, mi, half) in enumerate(row):
                    ksl = slice(kj * 128, (kj + 1) * 128)
                    new_ests = []
                    for hh in range(2):
                        st_ps = ps_fx.tile([128, 512], F32, tag="fx")
                        est = at.tile([128, 512], BF16, tag="est", bufs=4)
                        if half is None:
                            nc.tensor.matmul(
                                st_ps, kT_all[:, ksl],
                                qT_all[:, hh, pr], start=True, stop=True)
                            nc.scalar.activation(est, st_ps, AF.Exp)
                            if mi is not None:
                                for u in range(2):
                                    nc.vector.tensor_mul(
                                        est[:, u * 256:(u + 1) * 256],
                                        est[:, u * 256:(u + 1) * 256],
                                        maskt_sb[:, mi, :])
                        else:
                            hsl = slice(half * 128, half * 128 + 128)
                            nc.tensor.matmul(
                                st_ps[:, 0:256], kT_all[:, ksl],
                                qT_all[:, hh, pr, :, hsl],
                                start=True, stop=True)
                            nc.scalar.activation(est[:, 0:256],
                                                 st_ps[:, 0:256], AF.Exp)
                            for u in range(2):
                                nc.vector.tensor_mul(
                                    est[:, u * 128:(u + 1) * 128],
                                    est[:, u * 128:(u + 1) * 128],
                                    maskt_sb[:, mi, hsl])
                        new_ests.append(est)
                    if pending is not None:
                        flush_pv(*pending)
                    pending = (ki, kj, half, new_ests)
                flush_pv(*pending)
                if pr >= 1:
                    emit_wo(pr - 1)

                # unnormalized gated output (frees the ot banks early)
                for hh in range(2):
                    nc.vector.tensor_mul(
                        otg_ring[:, 2 * hh:2 * hh + 2, slot, :],
                        ot_ps[hh].rearrange("p (u q) -> p u q", u=2),
                        sig_ring[:, 2 * hh:2 * hh + 2, (pr // 2) % 2,
                                 (pr % 2) * 256:(pr % 2) * 256 + 256])
                # normalization: the broadcast rowsums are inverted on the
                # scalar engine (all 128 partitions in parallel)
                for hh in range(2):
                    rcp = at.tile([128, 512], FP16, tag=f"rcp{hh}", bufs=2)
                    _scalar_recip(nc, rcp, rs_ps[hh])
                    og = otg_ring[:, 2 * hh:2 * hh + 2, slot, :]
                    nc.vector.tensor_mul(
                        og, og, rcp.rearrange("p (u q) -> p u q", u=2))
            emit_wo(NT // 2 - 1, final=True)

    _split_excess_waits(nc)
    return nc


_CACHE = {}
LAST_EXEC_TIME_NS = None
LAST_RESULTS = None


def _maybe_install_profile_hook():
    if not os.environ.get("BASS_TRACE"):
        return
    try:
        import sys
        import types
        import antenv
        if "antenv.axon_hooks" in sys.modules:
            return
        mod = types.ModuleType("antenv.axon_hooks")
        mod._hook = None
        mod.set_axon_ntff_profile_hook = lambda h: setattr(mod, "_hook", h)
        mod.get_axon_ntff_profile_hook = lambda: mod._hook
        sys.modules["antenv.axon_hooks"] = mod
        antenv.axon_hooks = mod
        from trn_agent_boot.trn_boot import _ntff_profile_via_ctypes
        mod.set_axon_ntff_profile_hook(
            _ntff_profile_via_ctypes("/opt/axon/libaxon_pjrt.so"))
    except Exception:
        pass


def kernel(hidden_states, cos, sin, attention_mask, Wq, Wk, Wv, Wo, Wg,
           q_norm_w, k_norm_w):
    global LAST_EXEC_TIME_NS, LAST_RESULTS
    _maybe_install_profile_hook()

    hidden_states = np.asarray(hidden_states, dtype=np.float32)
    cos = np.asarray(cos, dtype=np.float32)
    sin = np.asarray(sin, dtype=np.float32)
    mask2d = np.asarray(attention_mask, dtype=np.float32).reshape(S, S)
    Wq = np.asarray(Wq, dtype=np.float32)
    Wk = np.asarray(Wk, dtype=np.float32)
    Wv = np.asarray(Wv, dtype=np.float32)
    Wo = np.asarray(Wo, dtype=np.float32)
    Wg = np.asarray(Wg, dtype=np.float32)
    qw = np.asarray(q_norm_w, dtype=np.float32)
    kw = np.asarray(k_norm_w, dtype=np.float32)

    import ml_dtypes
    rows, mixed = _mask_plan(mask2d)
    nmix = len(mixed)
    plan_key = (tuple(tuple(r) for r in rows), nmix)
    if plan_key not in _CACHE:
        _CACHE[plan_key] = _build(rows, nmix)
    nc = _CACHE[plan_key]

    sign = np.concatenate([-np.ones(D // 2), np.ones(D // 2)]).astype(np.float32)
    qw_swap = np.concatenate([qw[D // 2:], qw[:D // 2]])
    kw_swap = np.concatenate([kw[D // 2:], kw[:D // 2]])
    maskt_np = (np.concatenate(mixed, axis=0).astype(ml_dtypes.bfloat16)
                if nmix else None)  # [nmix*128, 256]

    in_maps = []
    for c in range(8):
        b, g = divmod(c, 4)
        qs = slice(g * DQ, (g + 1) * DQ)
        ks = slice(g * D, (g + 1) * D)
        xtT = np.ascontiguousarray(hidden_states[b].T)
        m = {
            "xt16": xtT.astype(np.float16),
            "wqkv": np.ascontiguousarray(
                np.concatenate([Wq[qs], Wk[ks], Wv[ks]],
                               axis=0).T).astype(np.float16),
            "wg": np.ascontiguousarray(Wg[qs].T).astype(np.float16),
            "wo": np.ascontiguousarray(Wo[:, qs].T).astype(np.float16),
            "csw": np.ascontiguousarray(np.concatenate([
                cos[b] * qw * LAM,
                sin[b] * (sign * qw_swap) * LAM,
                cos[b] * kw,
                sin[b] * (sign * kw_swap)], axis=1)).astype(np.float16),
        }
        if nmix:
            m["maskt"] = maskt_np
        in_maps.append(m)

    res = run_bass_kernel_spmd(nc, in_maps, list(range(8)),
                               trace=bool(os.environ.get("BASS_TRACE")))
    LAST_EXEC_TIME_NS = res.exec_time_ns
    LAST_RESULTS = res

    out = np.empty((B, S, H), dtype=np.float32)
    for b in range(B):
        acc = res.results[4 * b]["y"].astype(np.float32)
        for g in range(1, 4):
            acc = acc + res.results[4 * b + g]["y"]
        out[b] = acc
    return out
